# revision 52
# baseline (speedup 1.0000x reference)
"""Trainium2 Bass kernel for DialogueRNNCell (B=4096, T=128, P=2, D=256).

Strategy: data-parallel over batch across 8 cores (512 rows/core); no
cross-core communication.  Per core:
  - attention over g_hist streamed ONCE from HBM in [128t, 8b, 256d] 1MB
    chunks (the memory roofline): per-row scores via the fused DVE
    scalar_tensor_tensor (multiply by attn_w + free-axis reduce in one op),
    exp on the scalar engine, and the unnormalized context accumulated by
    per-row PE mini-matmuls (exp_s[:,b].T @ g[:,b,:], col-group tiled 4 per
    PSUM bank, bank-aligned outputs only — free-offset PSUM matmul outputs
    corrupt neighboring banks on this toolchain);
  - softmax normalization deferred to the end (1/l applied in batch-major),
    alpha transposed back per 128-row block during the stream;
  - three GRU cells computed feature-major on the PE (weights pre-transposed
    host-side, ih/hh partial sums fused in PSUM, biases pre-combined), with
    only the speaker slot of the party GRU evaluated (listener slots keep
    q0; the reference multiplies their output by the one-hot mask anyway).
Everything except the g_hist stream overlaps under the DMA: measured via
K-loop delta ~385 us/core, cost-model timeline 379 us, vs ~220 us pure-DMA
floor."""

import numpy as np

B, T, P = 4096, 128, 2
D = 256
NCORES = 8
BC = B // NCORES        # 512 rows per core
NI = BC // 128          # 4 partition tiles of batch
CHUNK_B = 8             # batch rows per streamed chunk
NCHUNK = BC // CHUNK_B  # 64

_PATCHED = False
_DEBUG = False
_NO_MINIS = False
_NO_SCORE = False
_HALF_STREAM = False
_NO_GRU = False
_NO_TAIL = False
LAST_RESULTS = None  # BassKernelResults of the most recent run (for test.py)


def _patch_tile_drain():
    """This container's walrus rejects >1 sem wait on one InstDrain
    ("Too many sync wait commands").  Spread the TileContext final-drain
    waits across single-wait NOPs instead."""
    global _PATCHED
    if _PATCHED:
        return
    import concourse.tile as tile
    from concourse import mybir
    from concourse.vector_clock import ScopedClock

    def _drain_and_barrier(self, tick_clock, wait_clock):
        nc = self.nc

        def spread(inst):
            si = inst.ins.sync_info
            if si is not None and si.on_wait and len(si.on_wait) > 1:
                waits = list(si.on_wait)
                si.on_wait = waits[:1]
                for w in waits[1:]:
                    n2 = nc.sync.nop()
                    n2.ins.sync_info = mybir.SyncInfo(on_wait=[w], on_update=[])

        probe = nc.sync.nop()
        wait_clock.add_sem_waits(
            probe.ins, ScopedClock({None: tick_clock.global_clock})
        )
        spread(probe)
        drain_inst = nc.sync.drain()
        wait_clock.add_sem_waits(
            drain_inst.ins, ScopedClock({None: tick_clock.global_clock})
        )
        spread(drain_inst)
        nc.all_engine_barrier()
        assert self.sems is not None
        popped = nc._tile_sem_poison_stack.pop()
        assert popped is self._sem_poison
        nc.clear_and_free_semaphores(list(self.sems.allocated().values()))
        nc.all_engine_barrier()

    tile.TileContext._drain_and_barrier = _drain_and_barrier
    _PATCHED = True


def _split_multi_waits(nc):
    """Walrus in this container encodes at most ONE sem wait per
    instruction.  Hoist extra waits onto same-engine NoOps placed
    immediately before the instruction in its basic block."""
    from concourse import mybir

    uid = 0
    for fn in nc.m.functions:
        for bb in fn.blocks:
            out = []
            changed = False
            for inst in bb.instructions:
                si = inst.sync_info
                if si is not None and si.on_wait and len(si.on_wait) > 1:
                    waits = list(si.on_wait)
                    for w in waits[:-1]:
                        nop = mybir.InstNoOp(name=f"I-waitsplit-{uid}",
                                             ins=[], outs=[])
                        uid += 1
                        nop.engine = inst.engine
                        nop.sync_info = mybir.SyncInfo(on_wait=[w],
                                                       on_update=[])
                        out.append(nop)
                    si.on_wait = waits[-1:]
                    changed = True
                out.append(inst)
            if changed:
                bb.instructions = out


def _build_program(loop_iters=None):
    import concourse.bass as bass
    import concourse.tile as tile
    from concourse import mybir
    from contextlib import nullcontext

    _patch_tile_drain()
    f32 = mybir.dt.float32
    Alu = mybir.AluOpType
    Act = mybir.ActivationFunctionType

    nc = bass.Bass("TRN2", target_bir_lowering=False, debug=False,
                   num_devices=NCORES)

    di = lambda name, shape: nc.declare_dram_parameter(name, list(shape), f32,
                                                       isOutput=False)
    do = lambda name, shape: nc.declare_dram_parameter(name, list(shape), f32,
                                                       isOutput=True)

    u_d = di("u", [BC, D])
    qmf_d = di("qmf", [128, NI])           # speaker index as f32, [p, i]
    g_d = di("g", [T, BC, D])
    q0_d = di("q0", [BC, P * D])
    e0_d = di("e0", [BC, D])
    wgi_d = di("wgi", [2 * D, 3 * D])      # wg_ih.T  [512, 768]
    wgh_d = di("wgh", [D, 3 * D])
    wpi_d = di("wpi", [2 * D, 3 * D])
    wph_d = di("wph", [D, 3 * D])
    wei_d = di("wei", [D, 3 * D])
    weh_d = di("weh", [D, 3 * D])
    bg_rz_d = di("bg_rz", [128, 4])        # summed ih+hh bias, feature-major
    bg_in_d = di("bg_in", [128, 2])
    bg_hn_d = di("bg_hn", [128, 2])
    bp_rz_d = di("bp_rz", [128, 4])
    bp_in_d = di("bp_in", [128, 2])
    bp_hn_d = di("bp_hn", [128, 2])
    be_rz_d = di("be_rz", [128, 4])
    be_in_d = di("be_in", [128, 2])
    be_hn_d = di("be_hn", [128, 2])
    aw_d = di("aw", [128, D])              # attn_w replicated over partitions
    id_d = di("ident", [128, 128])

    go_d = do("g_out", [BC, D])
    qo_d = do("q_out", [BC, P * D])
    eo_d = do("e_out", [BC, D])
    ao_d = do("a_out", [BC, T])
    dbg_d = {}
    if _DEBUG:
        for name, shape in [("qsel", [128, NI, D]), ("uT", [128, 2, BC]),
                            ("glT", [128, 2, BC]), ("cT", [128, 2, BC]),
                            ("rg", [128, 2, BC]), ("zg", [128, 2, BC]),
                            ("ng", [128, 2, BC]), ("gTd", [128, 2, BC]),
                            ("c_bd", [128, NI, D])]:
            dbg_d[name] = do("dbg_" + name, shape)

    with tile.TileContext(nc) as tc, \
         tc.tile_pool(name="const", bufs=1) as const, \
         tc.tile_pool(name="inb", bufs=1) as inb, \
         tc.tile_pool(name="fm", bufs=1) as fm, \
         tc.tile_pool(name="stream", bufs=4) as stream, \
         tc.tile_pool(name="sc", bufs=2) as sc, \
         tc.tile_pool(name="schunk", bufs=3) as schunk, \
         tc.tile_pool(name="stage", bufs=2) as stage, \
         tc.tile_pool(name="work", bufs=1) as work, \
         tc.tile_pool(name="gate", bufs=1) as gate, \
         tc.tile_pool(name="outb", bufs=1) as outb, \
         tc.tile_pool(name="cps", bufs=3, space="PSUM") as cps_pool, \
         tc.tile_pool(name="gps", bufs=3, space="PSUM") as gps_pool, \
         tc.tile_pool(name="tps", bufs=1, space="PSUM") as tps_pool, \
         (tc.For_i(0, loop_iters, 1) if loop_iters else nullcontext()):

        # ---- constants -------------------------------------------------
        wgi = const.tile([128, 4, 3 * D], f32)
        wgh = const.tile([128, 2, 3 * D], f32)
        wpi = const.tile([128, 4, 3 * D], f32)
        wph = const.tile([128, 2, 3 * D], f32)
        wei = const.tile([128, 2, 3 * D], f32)
        weh = const.tile([128, 2, 3 * D], f32)
        for _wt, _wd in [(wgi, wgi_d), (wgh, wgh_d), (wpi, wpi_d),
                         (wph, wph_d), (wei, wei_d), (weh, weh_d)]:
            nc.sync.dma_start(out=_wt, in_=_wd.ap().rearrange(
                "(k p) m -> p k m", p=128))

        def load_small(dram, shape, tag):
            t = const.tile(list(shape), f32, tag=tag)
            nc.sync.dma_start(out=t, in_=dram.ap())
            return t

        bg_rz = load_small(bg_rz_d, [128, 4], "bg_rz")
        bg_in = load_small(bg_in_d, [128, 2], "bg_in")
        bg_hn = load_small(bg_hn_d, [128, 2], "bg_hn")
        bp_rz = load_small(bp_rz_d, [128, 4], "bp_rz")
        bp_in = load_small(bp_in_d, [128, 2], "bp_in")
        bp_hn = load_small(bp_hn_d, [128, 2], "bp_hn")
        be_rz = load_small(be_rz_d, [128, 4], "be_rz")
        be_in = load_small(be_in_d, [128, 2], "be_in")
        be_hn = load_small(be_hn_d, [128, 2], "be_hn")
        attn_sb = load_small(aw_d, [128, D], "attn_sb")
        ident = load_small(id_d, [128, 128], "ident")
        qm = load_small(qmf_d, [128, NI], "qm")

        ones = const.tile([128, 128], f32)
        nc.vector.memset(ones, 1.0)
        qm1 = const.tile([128, NI], f32)   # 1 - qm
        nc.vector.tensor_scalar(out=qm1, in0=qm, scalar1=-1.0, scalar2=1.0,
                                op0=Alu.mult, op1=Alu.add)

        # ---- batch-major inputs ---------------------------------------
        u_b = inb.tile([128, NI, D], f32)
        nc.sync.dma_start(out=u_b, in_=u_d.ap().rearrange(
            "(i p) d -> p i d", p=128))
        q0_b = inb.tile([128, NI, P * D], f32)
        nc.sync.dma_start(out=q0_b, in_=q0_d.ap().rearrange(
            "(i p) d -> p i d", p=128))
        e0_b = inb.tile([128, NI, D], f32)
        nc.sync.dma_start(out=e0_b, in_=e0_d.ap().rearrange(
            "(i p) d -> p i d", p=128))
        gl_b = inb.tile([128, NI, D], f32)
        nc.sync.dma_start(out=gl_b, in_=g_d.ap()[T - 1].rearrange(
            "(i p) d -> p i d", p=128))

        # ---- speaker-slot select: q0_sel = q0[p0] + m*(q0[p1]-q0[p0]) --
        qsel_b = inb.tile([128, NI, D], f32)
        for i in range(NI):
            dif = work.tile([128, D], f32, tag="seldif")
            nc.vector.tensor_sub(dif, q0_b[:, i, D:2 * D], q0_b[:, i, 0:D])
            nc.vector.scalar_tensor_tensor(
                out=qsel_b[:, i, :], in0=dif, scalar=qm[:, i:i + 1],
                in1=q0_b[:, i, 0:D], op0=Alu.mult, op1=Alu.add)

        # ---- transposes to feature-major ------------------------------
        def to_fm(src_b, name):
            """[128, NI, D] batch-major -> [128, 2, BC] feature-major."""
            dst = fm.tile([128, 2, BC], f32, tag=name)
            for i in range(NI):
                for h in range(2):
                    tp = tps_pool.tile([128, 512], f32, tag="tp")
                    nc.tensor.transpose(tp[:, 0:128],
                                        src_b[:, i, h * 128:(h + 1) * 128],
                                        ident)
                    nc.scalar.copy(dst[:, h, i * 128:(i + 1) * 128],
                                   tp[:, 0:128])
            return dst

        uT = to_fm(u_b, "uT")
        qselT = to_fm(qsel_b, "qselT")
        e0T = to_fm(e0_b, "e0T")
        glT = to_fm(gl_b, "glT")

        # ---- attention stream over g_hist ------------------------------
        exp_s = fm.tile([128, BC], f32, tag="exp_s")   # [t, b] exp(score)
        exp_b = outb.tile([128, NI, T], f32, tag="exp_b")   # transposed exp
        c_b = inb.tile([128, NI, D], f32)              # unnormalized context
        if _NO_MINIS:
            nc.vector.memset(c_b, 0.0)
        if _HALF_STREAM:
            nc.vector.memset(exp_s, 1.0)
            nc.vector.memset(c_b, 0.0)
        for ci in range(NCHUNK // 2 if _HALF_STREAM else NCHUNK):
            chunk = stream.tile([128, CHUNK_B, D], f32, tag="chunk")
            nc.sync.dma_start(
                out=chunk, in_=g_d.ap()[:, ci * CHUNK_B:(ci + 1) * CHUNK_B, :])
            s_ch = schunk.tile([128, CHUNK_B], f32, tag="s_ch")
            if _NO_SCORE:
                nc.vector.memset(s_ch, 0.0)
            else:
                for j in range(CHUNK_B):
                    prod = sc.tile([128, D], f32, tag="prod")
                    nc.vector.scalar_tensor_tensor(
                        out=prod, in0=chunk[:, j, :], scalar=1.0, in1=attn_sb,
                        op0=Alu.mult, op1=Alu.mult,
                        accum_out=s_ch[:, j:j + 1])
            nc.scalar.activation(
                out=exp_s[:, ci * CHUNK_B:(ci + 1) * CHUNK_B], in_=s_ch,
                func=Act.Exp)
            if _NO_MINIS:
                continue
            for t4 in range(CHUNK_B // 4):   # 4 minis per PSUM bank
                cp = cps_pool.tile([128, D], f32, tag="cps")
                for jj in range(4):
                    j = t4 * 4 + jj
                    b = ci * CHUNK_B + j
                    nc.tensor.matmul(
                        cp[32 * jj:32 * jj + 1, :],
                        lhsT=exp_s[:, b:b + 1], rhs=chunk[:, j, :],
                        start=True, stop=True, tile_position=(0, 32 * jj))
                st = stage.tile([128, D], f32, tag="st")
                nc.scalar.copy(st, cp)   # rows 0/32/64/96 carry data
                b0 = ci * CHUNK_B + t4 * 4
                nc.sync.dma_start(
                    out=c_b[b0 % 128:b0 % 128 + 4, b0 // 128, :],
                    in_=st.rearrange("(jj r) d -> jj r d", jj=4)[:, 0])
            if (ci * CHUNK_B + CHUNK_B) % 128 == 0:
                # b-block i of exp_s complete: transpose it now (PE idle)
                i = (ci * CHUNK_B) // 128
                tp = tps_pool.tile([128, 512], f32, tag="tp")
                nc.tensor.transpose(tp[:, 0:128],
                                    exp_s[:, i * 128:(i + 1) * 128], ident)
                nc.scalar.copy(exp_b[:, i, :], tp[:, 0:128])

        # ---- softmax denominator -------------------------------------
        lp = tps_pool.tile([128, 512], f32, tag="lps")
        nc.tensor.matmul(lp[0:1, :], lhsT=ones[:, 0:1], rhs=exp_s,
                         start=True, stop=True)
        linv = work.tile([128, 512], f32, tag="linv")
        nc.vector.reciprocal(linv[0:1, :], lp[0:1, :])
        # 1/l to batch-major [128, NI] via tiny PE transposes
        linv_b = const.tile([128, NI], f32, tag="linv_b")
        for i in range(NI):
            tp = tps_pool.tile([128, 512], f32, tag="tp")
            # rank-1 "transpose": out[m, 0] = linv[0, i*128+m] * 1.0
            nc.tensor.matmul(tp[:, 0:1], lhsT=linv[0:1, i * 128:(i + 1) * 128],
                             rhs=ones[0:1, 0:1], start=True, stop=True)
            nc.scalar.copy(linv_b[:, i:i + 1], tp[:, 0:1])

        # ---- alpha: normalize pre-transposed exp in batch-major --------
        alpha_b = outb.tile([128, NI, T], f32, tag="alpha_b")
        for i in range(NI):
            nc.vector.tensor_scalar_mul(alpha_b[:, i, :], exp_b[:, i, :],
                                        linv_b[:, i:i + 1])
        nc.sync.dma_start(out=ao_d.ap().rearrange("(i p) d -> p i d", p=128),
                          in_=alpha_b)

        # ---- context: normalize in batch-major, then feature-major -----
        for i in range(NI):
            nc.vector.tensor_scalar_mul(c_b[:, i, :], c_b[:, i, :],
                                        linv_b[:, i:i + 1])
        cT = to_fm(c_b, "cT")

        # ---- GRU cells (feature-major) ---------------------------------
        def gru(xs, hs, wih, whh, b_rz, b_in, b_hn, name):
            out = fm.tile([128, 2, BC], f32, tag=name)
            r = gate.tile([128, 2, BC], f32, tag="r")
            z = gate.tile([128, 2, BC], f32, tag="z")
            n = gate.tile([128, 2, BC], f32, tag="n")
            for m in range(4):                    # r, z gates: ih + hh fused
                ps = gps_pool.tile([128, 512], f32, tag="gps")
                ops = [(wih, k, x) for k, x in enumerate(xs)] + \
                      [(whh, k, hh) for k, hh in enumerate(hs)]
                for idx, (w, k, rhs) in enumerate(ops):
                    nc.tensor.matmul(ps, lhsT=w[:, k, m * 128:(m + 1) * 128],
                                     rhs=rhs, start=(idx == 0),
                                     stop=(idx == len(ops) - 1))
                tgt = r if m < 2 else z
                nc.scalar.activation(out=tgt[:, m % 2, :], in_=ps,
                                     func=Act.Sigmoid,
                                     bias=b_rz[:, m:m + 1], scale=1.0)
            for mh in range(2):                   # n gate
                m = 4 + mh
                psi = gps_pool.tile([128, 512], f32, tag="gps")
                for k, x in enumerate(xs):
                    nc.tensor.matmul(psi, lhsT=wih[:, k, m * 128:(m + 1) * 128],
                                     rhs=x, start=(k == 0),
                                     stop=(k == len(xs) - 1))
                psh = gps_pool.tile([128, 512], f32, tag="gps")
                for k, hh in enumerate(hs):
                    nc.tensor.matmul(psh, lhsT=whh[:, k, m * 128:(m + 1) * 128],
                                     rhs=hh, start=(k == 0),
                                     stop=(k == len(hs) - 1))
                hnb = work.tile([128, BC], f32, tag="hnb")
                nc.scalar.activation(out=hnb, in_=psh, func=Act.Identity,
                                     bias=b_hn[:, mh:mh + 1], scale=1.0)
                t1 = work.tile([128, BC], f32, tag="t1")
                nc.vector.tensor_mul(t1, r[:, mh, :], hnb)
                t2 = work.tile([128, BC], f32, tag="t2")
                nc.vector.tensor_add(t2, t1, psi)
                nc.scalar.activation(out=n[:, mh, :], in_=t2, func=Act.Tanh,
                                     bias=b_in[:, mh:mh + 1], scale=1.0)
            for mh in range(2):                   # h' = n + z*(h - n)
                d1 = work.tile([128, BC], f32, tag="d1")
                nc.vector.tensor_sub(d1, hs[mh], n[:, mh, :])
                d2 = work.tile([128, BC], f32, tag="d2")
                nc.vector.tensor_mul(d2, z[:, mh, :], d1)
                nc.vector.tensor_add(out[:, mh, :], n[:, mh, :], d2)
            if _DEBUG and name == "gT":
                nc.sync.dma_start(out=dbg_d["rg"].ap(), in_=r)
                nc.sync.dma_start(out=dbg_d["zg"].ap(), in_=z)
                nc.sync.dma_start(out=dbg_d["ng"].ap(), in_=n)
                nc.sync.dma_start(out=dbg_d["gTd"].ap(), in_=out)
            return out

        if _DEBUG:
            nc.sync.dma_start(out=dbg_d["qsel"].ap(), in_=qsel_b)
            nc.sync.dma_start(out=dbg_d["uT"].ap(), in_=uT)
            nc.sync.dma_start(out=dbg_d["glT"].ap(), in_=glT)
            nc.sync.dma_start(out=dbg_d["cT"].ap(), in_=cT)
            nc.sync.dma_start(out=dbg_d["c_bd"].ap(), in_=c_b)

        if _NO_GRU:
            gT = qsT = eT = uT
        else:
            gT = gru([uT[:, 0], uT[:, 1], qselT[:, 0], qselT[:, 1]],
                     [glT[:, 0], glT[:, 1]], wgi, wgh, bg_rz, bg_in, bg_hn,
                     "gT")
            qsT = gru([uT[:, 0], uT[:, 1], cT[:, 0], cT[:, 1]],
                      [qselT[:, 0], qselT[:, 1]], wpi, wph, bp_rz, bp_in,
                      bp_hn, "qsT")
            eT = gru([qsT[:, 0], qsT[:, 1]], [e0T[:, 0], e0T[:, 1]],
                     wei, weh, be_rz, be_in, be_hn, "eT")

        # ---- back to batch-major + outputs -----------------------------
        def to_bm(src, name, width=D):
            dst = outb.tile([128, NI, width], f32, tag=name)
            for i in range(NI):
                for h in range(width // 128):
                    tp = tps_pool.tile([128, 512], f32, tag="tp")
                    nc.tensor.transpose(tp[:, 0:128],
                                        src[:, h, i * 128:(i + 1) * 128],
                                        ident)
                    nc.scalar.copy(dst[:, i, h * 128:(h + 1) * 128],
                                   tp[:, 0:128])
            return dst

        g_b = to_bm(gT, "g_b")
        qs_b = to_bm(qsT, "qs_b")
        e_b = to_bm(eT, "e_b")

        # q_ blend, in place over q0_b: speaker slot (p == qm) takes qs
        for i in range(NI):
            d1 = work.tile([128, D], f32, tag="qb1")
            nc.vector.tensor_sub(d1, q0_b[:, i, 0:D], qs_b[:, i, :])
            nc.vector.scalar_tensor_tensor(
                out=q0_b[:, i, 0:D], in0=d1, scalar=qm[:, i:i + 1],
                in1=qs_b[:, i, :], op0=Alu.mult, op1=Alu.add)
            d2 = work.tile([128, D], f32, tag="qb2")
            nc.vector.tensor_sub(d2, q0_b[:, i, D:2 * D], qs_b[:, i, :])
            nc.vector.scalar_tensor_tensor(
                out=q0_b[:, i, D:2 * D], in0=d2, scalar=qm1[:, i:i + 1],
                in1=qs_b[:, i, :], op0=Alu.mult, op1=Alu.add)

        nc.sync.dma_start(out=go_d.ap().rearrange("(i p) d -> p i d", p=128),
                          in_=g_b)
        nc.sync.dma_start(out=qo_d.ap().rearrange("(i p) d -> p i d", p=128),
                          in_=q0_b)
        nc.sync.dma_start(out=eo_d.ap().rearrange("(i p) d -> p i d", p=128),
                          in_=e_b)

    _split_multi_waits(nc)
    return nc


def kernel(U, qm_idx, g_hist, q0, e0,
           wg_ih, wg_hh, bg_ih, bg_hh,
           wp_ih, wp_hh, bp_ih, bp_hh,
           we_ih, we_hh, be_ih, be_hh,
           attn_w):
    global LAST_RESULTS
    from concourse.bass_utils import run_bass_kernel_spmd

    f = np.float32
    U = np.asarray(U, f)
    qm_idx = np.asarray(qm_idx)
    g_hist = np.asarray(g_hist, f)
    q0 = np.asarray(q0, f)
    e0 = np.asarray(e0, f)
    attn_w = np.asarray(attn_w, f)

    def wT(w):
        return np.ascontiguousarray(np.asarray(w, f).T)

    def bias_fm(v, lo, hi):
        v = np.asarray(v, f)[lo:hi]
        return np.ascontiguousarray(v.reshape(-1, 128).T)

    shared = {
        "wgi": wT(wg_ih), "wgh": wT(wg_hh),
        "wpi": wT(wp_ih), "wph": wT(wp_hh),
        "wei": wT(we_ih), "weh": wT(we_hh),
        "bg_rz": bias_fm(np.asarray(bg_ih, f) + np.asarray(bg_hh, f), 0, 512),
        "bg_in": bias_fm(bg_ih, 512, 768),
        "bg_hn": bias_fm(bg_hh, 512, 768),
        "bp_rz": bias_fm(np.asarray(bp_ih, f) + np.asarray(bp_hh, f), 0, 512),
        "bp_in": bias_fm(bp_ih, 512, 768),
        "bp_hn": bias_fm(bp_hh, 512, 768),
        "be_rz": bias_fm(np.asarray(be_ih, f) + np.asarray(be_hh, f), 0, 512),
        "be_in": bias_fm(be_ih, 512, 768),
        "be_hn": bias_fm(be_hh, 512, 768),
        "aw": np.ascontiguousarray(np.broadcast_to(attn_w.reshape(1, D),
                                                   (128, D))),
        "ident": np.eye(128, dtype=f),
    }

    qmf = qm_idx.astype(f)
    in_maps = []
    for c in range(NCORES):
        sl = slice(c * BC, (c + 1) * BC)
        m = dict(shared)
        m["u"] = U[sl]
        m["qmf"] = np.ascontiguousarray(qmf[sl].reshape(NI, 128).T)
        m["g"] = np.ascontiguousarray(g_hist[:, sl, :])
        m["q0"] = np.ascontiguousarray(q0[sl].reshape(BC, P * D))
        m["e0"] = e0[sl]
        in_maps.append(m)

    nc = _build_program()
    res = run_bass_kernel_spmd(nc, in_maps, list(range(NCORES)))
    LAST_RESULTS = res

    g_ = np.concatenate([res.results[c]["g_out"] for c in range(NCORES)], 0)
    q_ = np.concatenate([res.results[c]["q_out"] for c in range(NCORES)],
                        0).reshape(B, P, D)
    e_ = np.concatenate([res.results[c]["e_out"] for c in range(NCORES)], 0)
    alpha = np.concatenate([res.results[c]["a_out"] for c in range(NCORES)],
                           0).reshape(B, 1, T)
    return g_, q_, e_, alpha


# revision 54
# speedup vs baseline: 1.0028x; 1.0028x over previous
"""Trainium2 Bass kernel for DialogueRNNCell (B=4096, T=128, P=2, D=256).

Strategy: data-parallel over batch across 8 cores (512 rows/core); no
cross-core communication.  Per core:
  - attention over g_hist streamed ONCE from HBM in [128t, 8b, 256d] 1MB
    chunks (the memory roofline): per-row scores via the fused DVE
    scalar_tensor_tensor (multiply by attn_w + free-axis reduce in one op),
    exp on the scalar engine, and the unnormalized context accumulated by
    per-row PE mini-matmuls (exp_s[:,b].T @ g[:,b,:], col-group tiled 4 per
    PSUM bank, bank-aligned outputs only — free-offset PSUM matmul outputs
    corrupt neighboring banks on this toolchain);
  - softmax normalization deferred to the end (1/l applied in batch-major),
    alpha transposed back per 128-row block during the stream;
  - three GRU cells computed feature-major on the PE (weights pre-transposed
    host-side, ih/hh partial sums fused in PSUM, biases pre-combined), with
    only the speaker slot of the party GRU evaluated (listener slots keep
    q0; the reference multiplies their output by the one-hot mask anyway).
Everything except the g_hist stream overlaps under the DMA: measured via
K-loop delta ~385 us/core, cost-model timeline 379 us, vs ~220 us pure-DMA
floor."""

import numpy as np

B, T, P = 4096, 128, 2
D = 256
NCORES = 8
BC = B // NCORES        # 512 rows per core
NI = BC // 128          # 4 partition tiles of batch
CHUNK_B = 8             # batch rows per streamed chunk
NCHUNK = BC // CHUNK_B  # 64

_PATCHED = False
_DEBUG = False
_NO_MINIS = False
_NO_SCORE = False
_HALF_STREAM = False
_NO_GRU = False
_NO_TAIL = False
LAST_RESULTS = None  # BassKernelResults of the most recent run (for test.py)


def _patch_tile_drain():
    """This container's walrus rejects >1 sem wait on one InstDrain
    ("Too many sync wait commands").  Spread the TileContext final-drain
    waits across single-wait NOPs instead."""
    global _PATCHED
    if _PATCHED:
        return
    import concourse.tile as tile
    from concourse import mybir
    from concourse.vector_clock import ScopedClock

    def _drain_and_barrier(self, tick_clock, wait_clock):
        nc = self.nc

        def spread(inst):
            si = inst.ins.sync_info
            if si is not None and si.on_wait and len(si.on_wait) > 1:
                waits = list(si.on_wait)
                si.on_wait = waits[:1]
                for w in waits[1:]:
                    n2 = nc.sync.nop()
                    n2.ins.sync_info = mybir.SyncInfo(on_wait=[w], on_update=[])

        probe = nc.sync.nop()
        wait_clock.add_sem_waits(
            probe.ins, ScopedClock({None: tick_clock.global_clock})
        )
        spread(probe)
        drain_inst = nc.sync.drain()
        wait_clock.add_sem_waits(
            drain_inst.ins, ScopedClock({None: tick_clock.global_clock})
        )
        spread(drain_inst)
        nc.all_engine_barrier()
        assert self.sems is not None
        popped = nc._tile_sem_poison_stack.pop()
        assert popped is self._sem_poison
        nc.clear_and_free_semaphores(list(self.sems.allocated().values()))
        nc.all_engine_barrier()

    tile.TileContext._drain_and_barrier = _drain_and_barrier
    _PATCHED = True


def _split_multi_waits(nc):
    """Walrus in this container encodes at most ONE sem wait per
    instruction.  Hoist extra waits onto same-engine NoOps placed
    immediately before the instruction in its basic block."""
    from concourse import mybir

    uid = 0
    for fn in nc.m.functions:
        for bb in fn.blocks:
            out = []
            changed = False
            for inst in bb.instructions:
                si = inst.sync_info
                if si is not None and si.on_wait and len(si.on_wait) > 1:
                    waits = list(si.on_wait)
                    for w in waits[:-1]:
                        nop = mybir.InstNoOp(name=f"I-waitsplit-{uid}",
                                             ins=[], outs=[])
                        uid += 1
                        nop.engine = inst.engine
                        nop.sync_info = mybir.SyncInfo(on_wait=[w],
                                                       on_update=[])
                        out.append(nop)
                    si.on_wait = waits[-1:]
                    changed = True
                out.append(inst)
            if changed:
                bb.instructions = out


def _build_program(loop_iters=None):
    import concourse.bass as bass
    import concourse.tile as tile
    from concourse import mybir
    from contextlib import nullcontext

    _patch_tile_drain()
    f32 = mybir.dt.float32
    Alu = mybir.AluOpType
    Act = mybir.ActivationFunctionType

    nc = bass.Bass("TRN2", target_bir_lowering=False, debug=False,
                   num_devices=NCORES)

    di = lambda name, shape: nc.declare_dram_parameter(name, list(shape), f32,
                                                       isOutput=False)
    do = lambda name, shape: nc.declare_dram_parameter(name, list(shape), f32,
                                                       isOutput=True)

    u_d = di("u", [BC, D])
    qmf_d = di("qmf", [128, NI])           # speaker index as f32, [p, i]
    g_d = di("g", [T, BC, D])
    q0_d = di("q0", [BC, P * D])
    e0_d = di("e0", [BC, D])
    wgi_d = di("wgi", [2 * D, 3 * D])      # wg_ih.T  [512, 768]
    wgh_d = di("wgh", [D, 3 * D])
    wpi_d = di("wpi", [2 * D, 3 * D])
    wph_d = di("wph", [D, 3 * D])
    wei_d = di("wei", [D, 3 * D])
    weh_d = di("weh", [D, 3 * D])
    bg_rz_d = di("bg_rz", [128, 4])        # summed ih+hh bias, feature-major
    bg_in_d = di("bg_in", [128, 2])
    bg_hn_d = di("bg_hn", [128, 2])
    bp_rz_d = di("bp_rz", [128, 4])
    bp_in_d = di("bp_in", [128, 2])
    bp_hn_d = di("bp_hn", [128, 2])
    be_rz_d = di("be_rz", [128, 4])
    be_in_d = di("be_in", [128, 2])
    be_hn_d = di("be_hn", [128, 2])
    aw_d = di("aw", [128, D])              # attn_w replicated over partitions
    id_d = di("ident", [128, 128])

    go_d = do("g_out", [BC, D])
    qo_d = do("q_out", [BC, P * D])
    eo_d = do("e_out", [BC, D])
    ao_d = do("a_out", [BC, T])
    dbg_d = {}
    if _DEBUG:
        for name, shape in [("qsel", [128, NI, D]), ("uT", [128, 2, BC]),
                            ("glT", [128, 2, BC]), ("cT", [128, 2, BC]),
                            ("rg", [128, 2, BC]), ("zg", [128, 2, BC]),
                            ("ng", [128, 2, BC]), ("gTd", [128, 2, BC]),
                            ("c_bd", [128, NI, D])]:
            dbg_d[name] = do("dbg_" + name, shape)

    with tile.TileContext(nc) as tc, \
         tc.tile_pool(name="const", bufs=1) as const, \
         tc.tile_pool(name="inb", bufs=1) as inb, \
         tc.tile_pool(name="fm", bufs=1) as fm, \
         tc.tile_pool(name="stream", bufs=4) as stream, \
         tc.tile_pool(name="sc", bufs=2) as sc, \
         tc.tile_pool(name="schunk", bufs=3) as schunk, \
         tc.tile_pool(name="stage", bufs=2) as stage, \
         tc.tile_pool(name="work", bufs=1) as work, \
         tc.tile_pool(name="gate", bufs=1) as gate, \
         tc.tile_pool(name="outb", bufs=1) as outb, \
         tc.tile_pool(name="cps", bufs=3, space="PSUM") as cps_pool, \
         tc.tile_pool(name="gps", bufs=3, space="PSUM") as gps_pool, \
         tc.tile_pool(name="tps", bufs=2, space="PSUM") as tps_pool, \
         (tc.For_i(0, loop_iters, 1) if loop_iters else nullcontext()):

        # ---- constants -------------------------------------------------
        wgi = const.tile([128, 4, 3 * D], f32)
        wgh = const.tile([128, 2, 3 * D], f32)
        wpi = const.tile([128, 4, 3 * D], f32)
        wph = const.tile([128, 2, 3 * D], f32)
        wei = const.tile([128, 2, 3 * D], f32)
        weh = const.tile([128, 2, 3 * D], f32)
        for _wt, _wd in [(wgi, wgi_d), (wgh, wgh_d), (wpi, wpi_d),
                         (wph, wph_d), (wei, wei_d), (weh, weh_d)]:
            nc.sync.dma_start(out=_wt, in_=_wd.ap().rearrange(
                "(k p) m -> p k m", p=128))

        def load_small(dram, shape, tag):
            t = const.tile(list(shape), f32, tag=tag)
            nc.sync.dma_start(out=t, in_=dram.ap())
            return t

        bg_rz = load_small(bg_rz_d, [128, 4], "bg_rz")
        bg_in = load_small(bg_in_d, [128, 2], "bg_in")
        bg_hn = load_small(bg_hn_d, [128, 2], "bg_hn")
        bp_rz = load_small(bp_rz_d, [128, 4], "bp_rz")
        bp_in = load_small(bp_in_d, [128, 2], "bp_in")
        bp_hn = load_small(bp_hn_d, [128, 2], "bp_hn")
        be_rz = load_small(be_rz_d, [128, 4], "be_rz")
        be_in = load_small(be_in_d, [128, 2], "be_in")
        be_hn = load_small(be_hn_d, [128, 2], "be_hn")
        attn_sb = load_small(aw_d, [128, D], "attn_sb")
        ident = load_small(id_d, [128, 128], "ident")
        qm = load_small(qmf_d, [128, NI], "qm")

        ones = const.tile([128, 128], f32)
        nc.vector.memset(ones, 1.0)
        qm1 = const.tile([128, NI], f32)   # 1 - qm
        nc.vector.tensor_scalar(out=qm1, in0=qm, scalar1=-1.0, scalar2=1.0,
                                op0=Alu.mult, op1=Alu.add)

        # ---- batch-major inputs ---------------------------------------
        u_b = inb.tile([128, NI, D], f32)
        nc.sync.dma_start(out=u_b, in_=u_d.ap().rearrange(
            "(i p) d -> p i d", p=128))
        q0_b = inb.tile([128, NI, P * D], f32)
        nc.sync.dma_start(out=q0_b, in_=q0_d.ap().rearrange(
            "(i p) d -> p i d", p=128))
        e0_b = inb.tile([128, NI, D], f32)
        nc.sync.dma_start(out=e0_b, in_=e0_d.ap().rearrange(
            "(i p) d -> p i d", p=128))
        gl_b = inb.tile([128, NI, D], f32)
        nc.sync.dma_start(out=gl_b, in_=g_d.ap()[T - 1].rearrange(
            "(i p) d -> p i d", p=128))

        # ---- speaker-slot select: q0_sel = q0[p0] + m*(q0[p1]-q0[p0]) --
        qsel_b = inb.tile([128, NI, D], f32)
        for i in range(NI):
            dif = work.tile([128, D], f32, tag="seldif")
            nc.vector.tensor_sub(dif, q0_b[:, i, D:2 * D], q0_b[:, i, 0:D])
            nc.vector.scalar_tensor_tensor(
                out=qsel_b[:, i, :], in0=dif, scalar=qm[:, i:i + 1],
                in1=q0_b[:, i, 0:D], op0=Alu.mult, op1=Alu.add)

        # ---- transposes to feature-major ------------------------------
        def to_fm(src_b, name):
            """[128, NI, D] batch-major -> [128, 2, BC] feature-major."""
            dst = fm.tile([128, 2, BC], f32, tag=name)
            for i in range(NI):
                for h in range(2):
                    tp = tps_pool.tile([128, 512], f32, tag="tp")
                    nc.tensor.transpose(tp[:, 0:128],
                                        src_b[:, i, h * 128:(h + 1) * 128],
                                        ident)
                    nc.scalar.copy(dst[:, h, i * 128:(i + 1) * 128],
                                   tp[:, 0:128])
            return dst

        uT = to_fm(u_b, "uT")
        qselT = to_fm(qsel_b, "qselT")
        e0T = to_fm(e0_b, "e0T")
        glT = to_fm(gl_b, "glT")

        # ---- attention stream over g_hist ------------------------------
        exp_s = fm.tile([128, BC], f32, tag="exp_s")   # [t, b] exp(score)
        exp_b = outb.tile([128, NI, T], f32, tag="exp_b")   # transposed exp
        c_b = inb.tile([128, NI, D], f32)              # unnormalized context
        if _NO_MINIS:
            nc.vector.memset(c_b, 0.0)
        if _HALF_STREAM:
            nc.vector.memset(exp_s, 1.0)
            nc.vector.memset(c_b, 0.0)
        for ci in range(NCHUNK // 2 if _HALF_STREAM else NCHUNK):
            chunk = stream.tile([128, CHUNK_B, D], f32, tag="chunk")
            nc.sync.dma_start(
                out=chunk, in_=g_d.ap()[:, ci * CHUNK_B:(ci + 1) * CHUNK_B, :])
            s_ch = schunk.tile([128, CHUNK_B], f32, tag="s_ch")
            if _NO_SCORE:
                nc.vector.memset(s_ch, 0.0)
            else:
                for j in range(CHUNK_B):
                    prod = sc.tile([128, D], f32, tag="prod")
                    nc.vector.scalar_tensor_tensor(
                        out=prod, in0=chunk[:, j, :], scalar=1.0, in1=attn_sb,
                        op0=Alu.mult, op1=Alu.mult,
                        accum_out=s_ch[:, j:j + 1])
            nc.scalar.activation(
                out=exp_s[:, ci * CHUNK_B:(ci + 1) * CHUNK_B], in_=s_ch,
                func=Act.Exp)
            if _NO_MINIS:
                continue
            for t4 in range(CHUNK_B // 4):   # 4 minis per PSUM bank
                cp = cps_pool.tile([128, D], f32, tag="cps")
                for jj in range(4):
                    j = t4 * 4 + jj
                    b = ci * CHUNK_B + j
                    nc.tensor.matmul(
                        cp[32 * jj:32 * jj + 1, :],
                        lhsT=exp_s[:, b:b + 1], rhs=chunk[:, j, :],
                        start=True, stop=True, tile_position=(0, 32 * jj))
                st = stage.tile([128, D], f32, tag="st")
                nc.scalar.copy(st, cp)   # rows 0/32/64/96 carry data
                b0 = ci * CHUNK_B + t4 * 4
                nc.sync.dma_start(
                    out=c_b[b0 % 128:b0 % 128 + 4, b0 // 128, :],
                    in_=st.rearrange("(jj r) d -> jj r d", jj=4)[:, 0])
            if (ci * CHUNK_B + CHUNK_B) % 128 == 0:
                # b-block i of exp_s complete: transpose it now (PE idle)
                i = (ci * CHUNK_B) // 128
                tp = tps_pool.tile([128, 512], f32, tag="tp")
                nc.tensor.transpose(tp[:, 0:128],
                                    exp_s[:, i * 128:(i + 1) * 128], ident)
                nc.scalar.copy(exp_b[:, i, :], tp[:, 0:128])

        # ---- softmax denominator -------------------------------------
        lp = tps_pool.tile([128, 512], f32, tag="tp")
        nc.tensor.matmul(lp[0:1, :], lhsT=ones[:, 0:1], rhs=exp_s,
                         start=True, stop=True)
        linv = work.tile([128, 512], f32, tag="linv")
        nc.vector.reciprocal(linv[0:1, :], lp[0:1, :])
        # 1/l to batch-major [128, NI] via tiny PE transposes
        linv_b = const.tile([128, NI], f32, tag="linv_b")
        for i in range(NI):
            tp = tps_pool.tile([128, 512], f32, tag="tp")
            # rank-1 "transpose": out[m, 0] = linv[0, i*128+m] * 1.0
            nc.tensor.matmul(tp[:, 0:1], lhsT=linv[0:1, i * 128:(i + 1) * 128],
                             rhs=ones[0:1, 0:1], start=True, stop=True)
            nc.scalar.copy(linv_b[:, i:i + 1], tp[:, 0:1])

        # ---- alpha: normalize pre-transposed exp in batch-major --------
        alpha_b = outb.tile([128, NI, T], f32, tag="alpha_b")
        for i in range(NI):
            nc.vector.tensor_scalar_mul(alpha_b[:, i, :], exp_b[:, i, :],
                                        linv_b[:, i:i + 1])
        nc.sync.dma_start(out=ao_d.ap().rearrange("(i p) d -> p i d", p=128),
                          in_=alpha_b)

        # ---- context: normalize in batch-major, then feature-major -----
        for i in range(NI):
            nc.vector.tensor_scalar_mul(c_b[:, i, :], c_b[:, i, :],
                                        linv_b[:, i:i + 1])
        cT = to_fm(c_b, "cT")

        # ---- GRU cells (feature-major) ---------------------------------
        def gru(xs, hs, wih, whh, b_rz, b_in, b_hn, name):
            out = fm.tile([128, 2, BC], f32, tag=name)
            r = gate.tile([128, 2, BC], f32, tag="r")
            z = gate.tile([128, 2, BC], f32, tag="z")
            n = gate.tile([128, 2, BC], f32, tag="n")
            for m in range(4):                    # r, z gates: ih + hh fused
                ps = gps_pool.tile([128, 512], f32, tag="gps")
                ops = [(wih, k, x) for k, x in enumerate(xs)] + \
                      [(whh, k, hh) for k, hh in enumerate(hs)]
                for idx, (w, k, rhs) in enumerate(ops):
                    nc.tensor.matmul(ps, lhsT=w[:, k, m * 128:(m + 1) * 128],
                                     rhs=rhs, start=(idx == 0),
                                     stop=(idx == len(ops) - 1))
                tgt = r if m < 2 else z
                nc.scalar.activation(out=tgt[:, m % 2, :], in_=ps,
                                     func=Act.Sigmoid,
                                     bias=b_rz[:, m:m + 1], scale=1.0)
            for mh in range(2):                   # n gate
                m = 4 + mh
                psi = gps_pool.tile([128, 512], f32, tag="gps")
                for k, x in enumerate(xs):
                    nc.tensor.matmul(psi, lhsT=wih[:, k, m * 128:(m + 1) * 128],
                                     rhs=x, start=(k == 0),
                                     stop=(k == len(xs) - 1))
                psh = gps_pool.tile([128, 512], f32, tag="gps")
                for k, hh in enumerate(hs):
                    nc.tensor.matmul(psh, lhsT=whh[:, k, m * 128:(m + 1) * 128],
                                     rhs=hh, start=(k == 0),
                                     stop=(k == len(hs) - 1))
                hnb = work.tile([128, BC], f32, tag="hnb")
                nc.scalar.activation(out=hnb, in_=psh, func=Act.Identity,
                                     bias=b_hn[:, mh:mh + 1], scale=1.0)
                t1 = work.tile([128, BC], f32, tag="t1")
                nc.vector.tensor_mul(t1, r[:, mh, :], hnb)
                t2 = work.tile([128, BC], f32, tag="t2")
                nc.vector.tensor_add(t2, t1, psi)
                nc.scalar.activation(out=n[:, mh, :], in_=t2, func=Act.Tanh,
                                     bias=b_in[:, mh:mh + 1], scale=1.0)
            for mh in range(2):                   # h' = n + z*(h - n)
                d1 = work.tile([128, BC], f32, tag="d1")
                nc.vector.tensor_sub(d1, hs[mh], n[:, mh, :])
                d2 = work.tile([128, BC], f32, tag="d2")
                nc.vector.tensor_mul(d2, z[:, mh, :], d1)
                nc.vector.tensor_add(out[:, mh, :], n[:, mh, :], d2)
            if _DEBUG and name == "gT":
                nc.sync.dma_start(out=dbg_d["rg"].ap(), in_=r)
                nc.sync.dma_start(out=dbg_d["zg"].ap(), in_=z)
                nc.sync.dma_start(out=dbg_d["ng"].ap(), in_=n)
                nc.sync.dma_start(out=dbg_d["gTd"].ap(), in_=out)
            return out

        if _DEBUG:
            nc.sync.dma_start(out=dbg_d["qsel"].ap(), in_=qsel_b)
            nc.sync.dma_start(out=dbg_d["uT"].ap(), in_=uT)
            nc.sync.dma_start(out=dbg_d["glT"].ap(), in_=glT)
            nc.sync.dma_start(out=dbg_d["cT"].ap(), in_=cT)
            nc.sync.dma_start(out=dbg_d["c_bd"].ap(), in_=c_b)

        if _NO_GRU:
            gT = qsT = eT = uT
        else:
            gT = gru([uT[:, 0], uT[:, 1], qselT[:, 0], qselT[:, 1]],
                     [glT[:, 0], glT[:, 1]], wgi, wgh, bg_rz, bg_in, bg_hn,
                     "gT")
            qsT = gru([uT[:, 0], uT[:, 1], cT[:, 0], cT[:, 1]],
                      [qselT[:, 0], qselT[:, 1]], wpi, wph, bp_rz, bp_in,
                      bp_hn, "qsT")
            eT = gru([qsT[:, 0], qsT[:, 1]], [e0T[:, 0], e0T[:, 1]],
                     wei, weh, be_rz, be_in, be_hn, "eT")

        # ---- back to batch-major + outputs -----------------------------
        def to_bm(src, name, width=D):
            dst = outb.tile([128, NI, width], f32, tag=name)
            for i in range(NI):
                for h in range(width // 128):
                    tp = tps_pool.tile([128, 512], f32, tag="tp")
                    nc.tensor.transpose(tp[:, 0:128],
                                        src[:, h, i * 128:(i + 1) * 128],
                                        ident)
                    nc.scalar.copy(dst[:, i, h * 128:(h + 1) * 128],
                                   tp[:, 0:128])
            return dst

        g_b = to_bm(gT, "g_b")
        qs_b = to_bm(qsT, "qs_b")
        e_b = to_bm(eT, "e_b")

        # q_ blend, in place over q0_b: speaker slot (p == qm) takes qs
        for i in range(NI):
            d1 = work.tile([128, D], f32, tag="qb1")
            nc.vector.tensor_sub(d1, q0_b[:, i, 0:D], qs_b[:, i, :])
            nc.vector.scalar_tensor_tensor(
                out=q0_b[:, i, 0:D], in0=d1, scalar=qm[:, i:i + 1],
                in1=qs_b[:, i, :], op0=Alu.mult, op1=Alu.add)
            d2 = work.tile([128, D], f32, tag="qb2")
            nc.vector.tensor_sub(d2, q0_b[:, i, D:2 * D], qs_b[:, i, :])
            nc.vector.scalar_tensor_tensor(
                out=q0_b[:, i, D:2 * D], in0=d2, scalar=qm1[:, i:i + 1],
                in1=qs_b[:, i, :], op0=Alu.mult, op1=Alu.add)

        nc.sync.dma_start(out=go_d.ap().rearrange("(i p) d -> p i d", p=128),
                          in_=g_b)
        nc.sync.dma_start(out=qo_d.ap().rearrange("(i p) d -> p i d", p=128),
                          in_=q0_b)
        nc.sync.dma_start(out=eo_d.ap().rearrange("(i p) d -> p i d", p=128),
                          in_=e_b)

    _split_multi_waits(nc)
    return nc


def kernel(U, qm_idx, g_hist, q0, e0,
           wg_ih, wg_hh, bg_ih, bg_hh,
           wp_ih, wp_hh, bp_ih, bp_hh,
           we_ih, we_hh, be_ih, be_hh,
           attn_w):
    global LAST_RESULTS
    from concourse.bass_utils import run_bass_kernel_spmd

    f = np.float32
    U = np.asarray(U, f)
    qm_idx = np.asarray(qm_idx)
    g_hist = np.asarray(g_hist, f)
    q0 = np.asarray(q0, f)
    e0 = np.asarray(e0, f)
    attn_w = np.asarray(attn_w, f)

    def wT(w):
        return np.ascontiguousarray(np.asarray(w, f).T)

    def bias_fm(v, lo, hi):
        v = np.asarray(v, f)[lo:hi]
        return np.ascontiguousarray(v.reshape(-1, 128).T)

    shared = {
        "wgi": wT(wg_ih), "wgh": wT(wg_hh),
        "wpi": wT(wp_ih), "wph": wT(wp_hh),
        "wei": wT(we_ih), "weh": wT(we_hh),
        "bg_rz": bias_fm(np.asarray(bg_ih, f) + np.asarray(bg_hh, f), 0, 512),
        "bg_in": bias_fm(bg_ih, 512, 768),
        "bg_hn": bias_fm(bg_hh, 512, 768),
        "bp_rz": bias_fm(np.asarray(bp_ih, f) + np.asarray(bp_hh, f), 0, 512),
        "bp_in": bias_fm(bp_ih, 512, 768),
        "bp_hn": bias_fm(bp_hh, 512, 768),
        "be_rz": bias_fm(np.asarray(be_ih, f) + np.asarray(be_hh, f), 0, 512),
        "be_in": bias_fm(be_ih, 512, 768),
        "be_hn": bias_fm(be_hh, 512, 768),
        "aw": np.ascontiguousarray(np.broadcast_to(attn_w.reshape(1, D),
                                                   (128, D))),
        "ident": np.eye(128, dtype=f),
    }

    qmf = qm_idx.astype(f)
    in_maps = []
    for c in range(NCORES):
        sl = slice(c * BC, (c + 1) * BC)
        m = dict(shared)
        m["u"] = U[sl]
        m["qmf"] = np.ascontiguousarray(qmf[sl].reshape(NI, 128).T)
        m["g"] = np.ascontiguousarray(g_hist[:, sl, :])
        m["q0"] = np.ascontiguousarray(q0[sl].reshape(BC, P * D))
        m["e0"] = e0[sl]
        in_maps.append(m)

    nc = _build_program()
    res = run_bass_kernel_spmd(nc, in_maps, list(range(NCORES)))
    LAST_RESULTS = res

    g_ = np.concatenate([res.results[c]["g_out"] for c in range(NCORES)], 0)
    q_ = np.concatenate([res.results[c]["q_out"] for c in range(NCORES)],
                        0).reshape(B, P, D)
    e_ = np.concatenate([res.results[c]["e_out"] for c in range(NCORES)], 0)
    alpha = np.concatenate([res.results[c]["a_out"] for c in range(NCORES)],
                           0).reshape(B, 1, T)
    return g_, q_, e_, alpha


# revision 55
# speedup vs baseline: 1.0191x; 1.0163x over previous
"""Trainium2 Bass kernel for DialogueRNNCell (B=4096, T=128, P=2, D=256).

Strategy: data-parallel over batch across 8 cores (512 rows/core); no
cross-core communication.  Per core:
  - attention over g_hist streamed ONCE from HBM in [128t, 8b, 256d] 1MB
    chunks (the memory roofline): per-row scores via the fused DVE
    scalar_tensor_tensor (multiply by attn_w + free-axis reduce in one op),
    exp on the scalar engine, and the unnormalized context accumulated by
    per-row PE mini-matmuls (exp_s[:,b].T @ g[:,b,:], col-group tiled 4 per
    PSUM bank, bank-aligned outputs only — free-offset PSUM matmul outputs
    corrupt neighboring banks on this toolchain);
  - softmax normalization deferred to the end (1/l applied in batch-major),
    alpha transposed back per 128-row block during the stream;
  - three GRU cells computed feature-major on the PE (weights pre-transposed
    host-side, ih/hh partial sums fused in PSUM, biases pre-combined), with
    only the speaker slot of the party GRU evaluated (listener slots keep
    q0; the reference multiplies their output by the one-hot mask anyway).
Everything except the g_hist stream overlaps under the DMA: measured via
K-loop delta ~385 us/core, cost-model timeline 379 us, vs ~220 us pure-DMA
floor."""

import numpy as np

B, T, P = 4096, 128, 2
D = 256
NCORES = 8
BC = B // NCORES        # 512 rows per core
NI = BC // 128          # 4 partition tiles of batch
CHUNK_B = 8             # batch rows per streamed chunk
NCHUNK = BC // CHUNK_B  # 64

_PATCHED = False
_DEBUG = False
_NO_MINIS = False
_NO_SCORE = False
_HALF_STREAM = False
_NO_GRU = False
_NO_TAIL = False
LAST_RESULTS = None  # BassKernelResults of the most recent run (for test.py)


def _patch_tile_drain():
    """This container's walrus rejects >1 sem wait on one InstDrain
    ("Too many sync wait commands").  Spread the TileContext final-drain
    waits across single-wait NOPs instead."""
    global _PATCHED
    if _PATCHED:
        return
    import concourse.tile as tile
    from concourse import mybir
    from concourse.vector_clock import ScopedClock

    def _drain_and_barrier(self, tick_clock, wait_clock):
        nc = self.nc

        def spread(inst):
            si = inst.ins.sync_info
            if si is not None and si.on_wait and len(si.on_wait) > 1:
                waits = list(si.on_wait)
                si.on_wait = waits[:1]
                for w in waits[1:]:
                    n2 = nc.sync.nop()
                    n2.ins.sync_info = mybir.SyncInfo(on_wait=[w], on_update=[])

        probe = nc.sync.nop()
        wait_clock.add_sem_waits(
            probe.ins, ScopedClock({None: tick_clock.global_clock})
        )
        spread(probe)
        drain_inst = nc.sync.drain()
        wait_clock.add_sem_waits(
            drain_inst.ins, ScopedClock({None: tick_clock.global_clock})
        )
        spread(drain_inst)
        nc.all_engine_barrier()
        assert self.sems is not None
        popped = nc._tile_sem_poison_stack.pop()
        assert popped is self._sem_poison
        nc.clear_and_free_semaphores(list(self.sems.allocated().values()))
        nc.all_engine_barrier()

    tile.TileContext._drain_and_barrier = _drain_and_barrier
    _PATCHED = True


def _split_multi_waits(nc):
    """Walrus in this container encodes at most ONE sem wait per
    instruction.  Hoist extra waits onto same-engine NoOps placed
    immediately before the instruction in its basic block."""
    from concourse import mybir

    uid = 0
    for fn in nc.m.functions:
        for bb in fn.blocks:
            out = []
            changed = False
            for inst in bb.instructions:
                si = inst.sync_info
                if si is not None and si.on_wait and len(si.on_wait) > 1:
                    waits = list(si.on_wait)
                    for w in waits[:-1]:
                        nop = mybir.InstNoOp(name=f"I-waitsplit-{uid}",
                                             ins=[], outs=[])
                        uid += 1
                        nop.engine = inst.engine
                        nop.sync_info = mybir.SyncInfo(on_wait=[w],
                                                       on_update=[])
                        out.append(nop)
                    si.on_wait = waits[-1:]
                    changed = True
                out.append(inst)
            if changed:
                bb.instructions = out


def _build_program(loop_iters=None):
    import concourse.bass as bass
    import concourse.tile as tile
    from concourse import mybir
    from contextlib import nullcontext

    _patch_tile_drain()
    f32 = mybir.dt.float32
    Alu = mybir.AluOpType
    Act = mybir.ActivationFunctionType

    nc = bass.Bass("TRN2", target_bir_lowering=False, debug=False,
                   num_devices=NCORES)

    di = lambda name, shape: nc.declare_dram_parameter(name, list(shape), f32,
                                                       isOutput=False)
    do = lambda name, shape: nc.declare_dram_parameter(name, list(shape), f32,
                                                       isOutput=True)

    u_d = di("u", [BC, D])
    qmf_d = di("qmf", [128, NI])           # speaker index as f32, [p, i]
    g_d = di("g", [T, BC, D])
    q0_d = di("q0", [BC, P * D])
    e0_d = di("e0", [BC, D])
    wgi_d = di("wgi", [2 * D, 3 * D])      # wg_ih.T  [512, 768]
    wgh_d = di("wgh", [D, 3 * D])
    wpi_d = di("wpi", [2 * D, 3 * D])
    wph_d = di("wph", [D, 3 * D])
    wei_d = di("wei", [D, 3 * D])
    weh_d = di("weh", [D, 3 * D])
    bg_rz_d = di("bg_rz", [128, 4])        # summed ih+hh bias, feature-major
    bg_in_d = di("bg_in", [128, 2])
    bg_hn_d = di("bg_hn", [128, 2])
    bp_rz_d = di("bp_rz", [128, 4])
    bp_in_d = di("bp_in", [128, 2])
    bp_hn_d = di("bp_hn", [128, 2])
    be_rz_d = di("be_rz", [128, 4])
    be_in_d = di("be_in", [128, 2])
    be_hn_d = di("be_hn", [128, 2])
    aw_d = di("aw", [128, D])              # attn_w replicated over partitions
    id_d = di("ident", [128, 128])

    go_d = do("g_out", [BC, D])
    qo_d = do("q_out", [BC, P * D])
    eo_d = do("e_out", [BC, D])
    ao_d = do("a_out", [BC, T])
    dbg_d = {}
    if _DEBUG:
        for name, shape in [("qsel", [128, NI, D]), ("uT", [128, 2, BC]),
                            ("glT", [128, 2, BC]), ("cT", [128, 2, BC]),
                            ("rg", [128, 2, BC]), ("zg", [128, 2, BC]),
                            ("ng", [128, 2, BC]), ("gTd", [128, 2, BC]),
                            ("c_bd", [128, NI, D])]:
            dbg_d[name] = do("dbg_" + name, shape)

    with tile.TileContext(nc) as tc, \
         tc.tile_pool(name="const", bufs=1) as const, \
         tc.tile_pool(name="inb", bufs=1) as inb, \
         tc.tile_pool(name="fm", bufs=1) as fm, \
         tc.tile_pool(name="stream", bufs=5) as stream, \
         tc.tile_pool(name="sc", bufs=2) as sc, \
         tc.tile_pool(name="schunk", bufs=3) as schunk, \
         tc.tile_pool(name="stage", bufs=2) as stage, \
         tc.tile_pool(name="work", bufs=1) as work, \
         tc.tile_pool(name="gate", bufs=1) as gate, \
         tc.tile_pool(name="outb", bufs=1) as outb, \
         tc.tile_pool(name="cps", bufs=3, space="PSUM") as cps_pool, \
         tc.tile_pool(name="gps", bufs=3, space="PSUM") as gps_pool, \
         tc.tile_pool(name="tps", bufs=2, space="PSUM") as tps_pool, \
         (tc.For_i(0, loop_iters, 1) if loop_iters else nullcontext()):

        # ---- constants -------------------------------------------------
        wgi = const.tile([128, 4, 3 * D], f32)
        wgh = const.tile([128, 2, 3 * D], f32)
        wpi = const.tile([128, 4, 3 * D], f32)
        wph = const.tile([128, 2, 3 * D], f32)
        wei = const.tile([128, 2, 3 * D], f32)
        weh = const.tile([128, 2, 3 * D], f32)
        for _wt, _wd in [(wgi, wgi_d), (wgh, wgh_d), (wpi, wpi_d),
                         (wph, wph_d), (wei, wei_d), (weh, weh_d)]:
            nc.sync.dma_start(out=_wt, in_=_wd.ap().rearrange(
                "(k p) m -> p k m", p=128))

        def load_small(dram, shape, tag):
            t = const.tile(list(shape), f32, tag=tag)
            nc.sync.dma_start(out=t, in_=dram.ap())
            return t

        bg_rz = load_small(bg_rz_d, [128, 4], "bg_rz")
        bg_in = load_small(bg_in_d, [128, 2], "bg_in")
        bg_hn = load_small(bg_hn_d, [128, 2], "bg_hn")
        bp_rz = load_small(bp_rz_d, [128, 4], "bp_rz")
        bp_in = load_small(bp_in_d, [128, 2], "bp_in")
        bp_hn = load_small(bp_hn_d, [128, 2], "bp_hn")
        be_rz = load_small(be_rz_d, [128, 4], "be_rz")
        be_in = load_small(be_in_d, [128, 2], "be_in")
        be_hn = load_small(be_hn_d, [128, 2], "be_hn")
        attn_sb = load_small(aw_d, [128, D], "attn_sb")
        ident = load_small(id_d, [128, 128], "ident")
        qm = load_small(qmf_d, [128, NI], "qm")

        ones = const.tile([128, 128], f32)
        nc.vector.memset(ones, 1.0)
        qm1 = const.tile([128, NI], f32)   # 1 - qm
        nc.vector.tensor_scalar(out=qm1, in0=qm, scalar1=-1.0, scalar2=1.0,
                                op0=Alu.mult, op1=Alu.add)

        # ---- batch-major inputs ---------------------------------------
        u_b = inb.tile([128, NI, D], f32)
        nc.sync.dma_start(out=u_b, in_=u_d.ap().rearrange(
            "(i p) d -> p i d", p=128))
        q0_b = inb.tile([128, NI, P * D], f32)
        nc.sync.dma_start(out=q0_b, in_=q0_d.ap().rearrange(
            "(i p) d -> p i d", p=128))
        e0_b = inb.tile([128, NI, D], f32)
        nc.sync.dma_start(out=e0_b, in_=e0_d.ap().rearrange(
            "(i p) d -> p i d", p=128))
        gl_b = inb.tile([128, NI, D], f32)
        nc.sync.dma_start(out=gl_b, in_=g_d.ap()[T - 1].rearrange(
            "(i p) d -> p i d", p=128))

        # ---- speaker-slot select: q0_sel = q0[p0] + m*(q0[p1]-q0[p0]) --
        qsel_b = inb.tile([128, NI, D], f32)
        for i in range(NI):
            dif = work.tile([128, D], f32, tag="seldif")
            nc.vector.tensor_sub(dif, q0_b[:, i, D:2 * D], q0_b[:, i, 0:D])
            nc.vector.scalar_tensor_tensor(
                out=qsel_b[:, i, :], in0=dif, scalar=qm[:, i:i + 1],
                in1=q0_b[:, i, 0:D], op0=Alu.mult, op1=Alu.add)

        # ---- transposes to feature-major ------------------------------
        def to_fm(src_b, name):
            """[128, NI, D] batch-major -> [128, 2, BC] feature-major."""
            dst = fm.tile([128, 2, BC], f32, tag=name)
            for i in range(NI):
                for h in range(2):
                    tp = tps_pool.tile([128, 512], f32, tag="tp")
                    nc.tensor.transpose(tp[:, 0:128],
                                        src_b[:, i, h * 128:(h + 1) * 128],
                                        ident)
                    nc.scalar.copy(dst[:, h, i * 128:(i + 1) * 128],
                                   tp[:, 0:128])
            return dst

        uT = to_fm(u_b, "uT")
        qselT = to_fm(qsel_b, "qselT")
        e0T = to_fm(e0_b, "e0T")
        glT = to_fm(gl_b, "glT")

        # ---- attention stream over g_hist ------------------------------
        exp_s = fm.tile([128, BC], f32, tag="exp_s")   # [t, b] exp(score)
        exp_b = outb.tile([128, NI, T], f32, tag="exp_b")   # transposed exp
        c_b = inb.tile([128, NI, D], f32)              # unnormalized context
        if _NO_MINIS:
            nc.vector.memset(c_b, 0.0)
        if _HALF_STREAM:
            nc.vector.memset(exp_s, 1.0)
            nc.vector.memset(c_b, 0.0)
        for ci in range(NCHUNK // 2 if _HALF_STREAM else NCHUNK):
            chunk = stream.tile([128, CHUNK_B, D], f32, tag="chunk")
            nc.sync.dma_start(
                out=chunk, in_=g_d.ap()[:, ci * CHUNK_B:(ci + 1) * CHUNK_B, :])
            s_ch = schunk.tile([128, CHUNK_B], f32, tag="s_ch")
            if _NO_SCORE:
                nc.vector.memset(s_ch, 0.0)
            else:
                for j in range(CHUNK_B):
                    prod = sc.tile([128, D], f32, tag="prod")
                    nc.vector.scalar_tensor_tensor(
                        out=prod, in0=chunk[:, j, :], scalar=1.0, in1=attn_sb,
                        op0=Alu.mult, op1=Alu.mult,
                        accum_out=s_ch[:, j:j + 1])
            nc.scalar.activation(
                out=exp_s[:, ci * CHUNK_B:(ci + 1) * CHUNK_B], in_=s_ch,
                func=Act.Exp)
            if _NO_MINIS:
                continue
            for t4 in range(CHUNK_B // 4):   # 4 minis per PSUM bank
                cp = cps_pool.tile([128, D], f32, tag="cps")
                for jj in range(4):
                    j = t4 * 4 + jj
                    b = ci * CHUNK_B + j
                    nc.tensor.matmul(
                        cp[32 * jj:32 * jj + 1, :],
                        lhsT=exp_s[:, b:b + 1], rhs=chunk[:, j, :],
                        start=True, stop=True, tile_position=(0, 32 * jj))
                st = stage.tile([128, D], f32, tag="st")
                nc.scalar.copy(st, cp)   # rows 0/32/64/96 carry data
                b0 = ci * CHUNK_B + t4 * 4
                nc.sync.dma_start(
                    out=c_b[b0 % 128:b0 % 128 + 4, b0 // 128, :],
                    in_=st.rearrange("(jj r) d -> jj r d", jj=4)[:, 0])
            if (ci * CHUNK_B + CHUNK_B) % 128 == 0:
                # b-block i of exp_s complete: transpose it now (PE idle)
                i = (ci * CHUNK_B) // 128
                tp = tps_pool.tile([128, 512], f32, tag="tp")
                nc.tensor.transpose(tp[:, 0:128],
                                    exp_s[:, i * 128:(i + 1) * 128], ident)
                nc.scalar.copy(exp_b[:, i, :], tp[:, 0:128])

        # ---- softmax denominator -------------------------------------
        lp = tps_pool.tile([128, 512], f32, tag="tp")
        nc.tensor.matmul(lp[0:1, :], lhsT=ones[:, 0:1], rhs=exp_s,
                         start=True, stop=True)
        linv = work.tile([128, 512], f32, tag="linv")
        nc.vector.reciprocal(linv[0:1, :], lp[0:1, :])
        # 1/l to batch-major [128, NI] via tiny PE transposes
        linv_b = const.tile([128, NI], f32, tag="linv_b")
        for i in range(NI):
            tp = tps_pool.tile([128, 512], f32, tag="tp")
            # rank-1 "transpose": out[m, 0] = linv[0, i*128+m] * 1.0
            nc.tensor.matmul(tp[:, 0:1], lhsT=linv[0:1, i * 128:(i + 1) * 128],
                             rhs=ones[0:1, 0:1], start=True, stop=True)
            nc.scalar.copy(linv_b[:, i:i + 1], tp[:, 0:1])

        # ---- alpha: normalize pre-transposed exp in batch-major --------
        alpha_b = outb.tile([128, NI, T], f32, tag="alpha_b")
        for i in range(NI):
            nc.vector.tensor_scalar_mul(alpha_b[:, i, :], exp_b[:, i, :],
                                        linv_b[:, i:i + 1])
        nc.sync.dma_start(out=ao_d.ap().rearrange("(i p) d -> p i d", p=128),
                          in_=alpha_b)

        # ---- context: normalize in batch-major, then feature-major -----
        for i in range(NI):
            nc.vector.tensor_scalar_mul(c_b[:, i, :], c_b[:, i, :],
                                        linv_b[:, i:i + 1])
        cT = to_fm(c_b, "cT")

        # ---- GRU cells (feature-major) ---------------------------------
        def gru(xs, hs, wih, whh, b_rz, b_in, b_hn, name):
            out = fm.tile([128, 2, BC], f32, tag=name)
            r = gate.tile([128, 2, BC], f32, tag="r")
            z = gate.tile([128, 2, BC], f32, tag="z")
            n = gate.tile([128, 2, BC], f32, tag="n")
            for m in range(4):                    # r, z gates: ih + hh fused
                ps = gps_pool.tile([128, 512], f32, tag="gps")
                ops = [(wih, k, x) for k, x in enumerate(xs)] + \
                      [(whh, k, hh) for k, hh in enumerate(hs)]
                for idx, (w, k, rhs) in enumerate(ops):
                    nc.tensor.matmul(ps, lhsT=w[:, k, m * 128:(m + 1) * 128],
                                     rhs=rhs, start=(idx == 0),
                                     stop=(idx == len(ops) - 1))
                tgt = r if m < 2 else z
                nc.scalar.activation(out=tgt[:, m % 2, :], in_=ps,
                                     func=Act.Sigmoid,
                                     bias=b_rz[:, m:m + 1], scale=1.0)
            for mh in range(2):                   # n gate
                m = 4 + mh
                psi = gps_pool.tile([128, 512], f32, tag="gps")
                for k, x in enumerate(xs):
                    nc.tensor.matmul(psi, lhsT=wih[:, k, m * 128:(m + 1) * 128],
                                     rhs=x, start=(k == 0),
                                     stop=(k == len(xs) - 1))
                psh = gps_pool.tile([128, 512], f32, tag="gps")
                for k, hh in enumerate(hs):
                    nc.tensor.matmul(psh, lhsT=whh[:, k, m * 128:(m + 1) * 128],
                                     rhs=hh, start=(k == 0),
                                     stop=(k == len(hs) - 1))
                hnb = work.tile([128, BC], f32, tag="hnb")
                nc.scalar.activation(out=hnb, in_=psh, func=Act.Identity,
                                     bias=b_hn[:, mh:mh + 1], scale=1.0)
                t1 = work.tile([128, BC], f32, tag="t1")
                nc.vector.tensor_mul(t1, r[:, mh, :], hnb)
                t2 = work.tile([128, BC], f32, tag="t2")
                nc.vector.tensor_add(t2, t1, psi)
                nc.scalar.activation(out=n[:, mh, :], in_=t2, func=Act.Tanh,
                                     bias=b_in[:, mh:mh + 1], scale=1.0)
            for mh in range(2):                   # h' = n + z*(h - n)
                d1 = work.tile([128, BC], f32, tag="d1")
                nc.vector.tensor_sub(d1, hs[mh], n[:, mh, :])
                d2 = work.tile([128, BC], f32, tag="d2")
                nc.vector.tensor_mul(d2, z[:, mh, :], d1)
                nc.vector.tensor_add(out[:, mh, :], n[:, mh, :], d2)
            if _DEBUG and name == "gT":
                nc.sync.dma_start(out=dbg_d["rg"].ap(), in_=r)
                nc.sync.dma_start(out=dbg_d["zg"].ap(), in_=z)
                nc.sync.dma_start(out=dbg_d["ng"].ap(), in_=n)
                nc.sync.dma_start(out=dbg_d["gTd"].ap(), in_=out)
            return out

        if _DEBUG:
            nc.sync.dma_start(out=dbg_d["qsel"].ap(), in_=qsel_b)
            nc.sync.dma_start(out=dbg_d["uT"].ap(), in_=uT)
            nc.sync.dma_start(out=dbg_d["glT"].ap(), in_=glT)
            nc.sync.dma_start(out=dbg_d["cT"].ap(), in_=cT)
            nc.sync.dma_start(out=dbg_d["c_bd"].ap(), in_=c_b)

        if _NO_GRU:
            gT = qsT = eT = uT
        else:
            gT = gru([uT[:, 0], uT[:, 1], qselT[:, 0], qselT[:, 1]],
                     [glT[:, 0], glT[:, 1]], wgi, wgh, bg_rz, bg_in, bg_hn,
                     "gT")
            qsT = gru([uT[:, 0], uT[:, 1], cT[:, 0], cT[:, 1]],
                      [qselT[:, 0], qselT[:, 1]], wpi, wph, bp_rz, bp_in,
                      bp_hn, "qsT")
            eT = gru([qsT[:, 0], qsT[:, 1]], [e0T[:, 0], e0T[:, 1]],
                     wei, weh, be_rz, be_in, be_hn, "eT")

        # ---- back to batch-major + outputs -----------------------------
        def to_bm(src, name, width=D):
            dst = outb.tile([128, NI, width], f32, tag=name)
            for i in range(NI):
                for h in range(width // 128):
                    tp = tps_pool.tile([128, 512], f32, tag="tp")
                    nc.tensor.transpose(tp[:, 0:128],
                                        src[:, h, i * 128:(i + 1) * 128],
                                        ident)
                    nc.scalar.copy(dst[:, i, h * 128:(h + 1) * 128],
                                   tp[:, 0:128])
            return dst

        g_b = to_bm(gT, "g_b")
        qs_b = to_bm(qsT, "qs_b")
        e_b = to_bm(eT, "e_b")

        # q_ blend, in place over q0_b: speaker slot (p == qm) takes qs
        for i in range(NI):
            d1 = work.tile([128, D], f32, tag="qb1")
            nc.vector.tensor_sub(d1, q0_b[:, i, 0:D], qs_b[:, i, :])
            nc.vector.scalar_tensor_tensor(
                out=q0_b[:, i, 0:D], in0=d1, scalar=qm[:, i:i + 1],
                in1=qs_b[:, i, :], op0=Alu.mult, op1=Alu.add)
            d2 = work.tile([128, D], f32, tag="qb2")
            nc.vector.tensor_sub(d2, q0_b[:, i, D:2 * D], qs_b[:, i, :])
            nc.vector.scalar_tensor_tensor(
                out=q0_b[:, i, D:2 * D], in0=d2, scalar=qm1[:, i:i + 1],
                in1=qs_b[:, i, :], op0=Alu.mult, op1=Alu.add)

        nc.sync.dma_start(out=go_d.ap().rearrange("(i p) d -> p i d", p=128),
                          in_=g_b)
        nc.sync.dma_start(out=qo_d.ap().rearrange("(i p) d -> p i d", p=128),
                          in_=q0_b)
        nc.sync.dma_start(out=eo_d.ap().rearrange("(i p) d -> p i d", p=128),
                          in_=e_b)

    _split_multi_waits(nc)
    return nc


def kernel(U, qm_idx, g_hist, q0, e0,
           wg_ih, wg_hh, bg_ih, bg_hh,
           wp_ih, wp_hh, bp_ih, bp_hh,
           we_ih, we_hh, be_ih, be_hh,
           attn_w):
    global LAST_RESULTS
    from concourse.bass_utils import run_bass_kernel_spmd

    f = np.float32
    U = np.asarray(U, f)
    qm_idx = np.asarray(qm_idx)
    g_hist = np.asarray(g_hist, f)
    q0 = np.asarray(q0, f)
    e0 = np.asarray(e0, f)
    attn_w = np.asarray(attn_w, f)

    def wT(w):
        return np.ascontiguousarray(np.asarray(w, f).T)

    def bias_fm(v, lo, hi):
        v = np.asarray(v, f)[lo:hi]
        return np.ascontiguousarray(v.reshape(-1, 128).T)

    shared = {
        "wgi": wT(wg_ih), "wgh": wT(wg_hh),
        "wpi": wT(wp_ih), "wph": wT(wp_hh),
        "wei": wT(we_ih), "weh": wT(we_hh),
        "bg_rz": bias_fm(np.asarray(bg_ih, f) + np.asarray(bg_hh, f), 0, 512),
        "bg_in": bias_fm(bg_ih, 512, 768),
        "bg_hn": bias_fm(bg_hh, 512, 768),
        "bp_rz": bias_fm(np.asarray(bp_ih, f) + np.asarray(bp_hh, f), 0, 512),
        "bp_in": bias_fm(bp_ih, 512, 768),
        "bp_hn": bias_fm(bp_hh, 512, 768),
        "be_rz": bias_fm(np.asarray(be_ih, f) + np.asarray(be_hh, f), 0, 512),
        "be_in": bias_fm(be_ih, 512, 768),
        "be_hn": bias_fm(be_hh, 512, 768),
        "aw": np.ascontiguousarray(np.broadcast_to(attn_w.reshape(1, D),
                                                   (128, D))),
        "ident": np.eye(128, dtype=f),
    }

    qmf = qm_idx.astype(f)
    in_maps = []
    for c in range(NCORES):
        sl = slice(c * BC, (c + 1) * BC)
        m = dict(shared)
        m["u"] = U[sl]
        m["qmf"] = np.ascontiguousarray(qmf[sl].reshape(NI, 128).T)
        m["g"] = np.ascontiguousarray(g_hist[:, sl, :])
        m["q0"] = np.ascontiguousarray(q0[sl].reshape(BC, P * D))
        m["e0"] = e0[sl]
        in_maps.append(m)

    nc = _build_program()
    res = run_bass_kernel_spmd(nc, in_maps, list(range(NCORES)))
    LAST_RESULTS = res

    g_ = np.concatenate([res.results[c]["g_out"] for c in range(NCORES)], 0)
    q_ = np.concatenate([res.results[c]["q_out"] for c in range(NCORES)],
                        0).reshape(B, P, D)
    e_ = np.concatenate([res.results[c]["e_out"] for c in range(NCORES)], 0)
    alpha = np.concatenate([res.results[c]["a_out"] for c in range(NCORES)],
                           0).reshape(B, 1, T)
    return g_, q_, e_, alpha
